# revision 4
# baseline (speedup 1.0000x reference)
"""Multi-head attention (B=4, S=2048, D=1024, H=16, causal) on 8 Trainium2
NeuronCores via Bass/Tile.

Sharding: core c handles batch b=c//2 and head-group hg=c%2 (8 heads each).
Each core computes its QKV projections (column-sharded weights), causal
attention with full attention-probability output, and a partial output
projection (summed across the 2 head-groups on the host during unshard).
"""

import sys

sys.path.insert(0, "/opt/trn_rl_repo")

import numpy as np
import ml_dtypes

D_MODEL = 1024
NUM_HEADS = 16
D_K = 64
B = 4
S = 2048
N_CORES = 8
H_LOC = 8          # heads per core
DP = H_LOC * D_K   # 512 local projection dims per core
SCALE = 1.0 / np.sqrt(D_K)
NEG = -1.0e30

BF16 = ml_dtypes.bfloat16

_cache = {}


def _build_nc():
    import concourse.tile as tile
    from concourse import bacc, mybir

    F32 = mybir.dt.float32
    BF = mybir.dt.bfloat16
    EXP = mybir.ActivationFunctionType.Exp
    IDENT = mybir.ActivationFunctionType.Identity

    nc = bacc.Bacc("TRN2", target_bir_lowering=False, debug=False,
                   num_devices=N_CORES)

    XqT = nc.dram_tensor("XqT", [D_MODEL, S], BF, kind="ExternalInput")
    XkT = nc.dram_tensor("XkT", [D_MODEL, S], BF, kind="ExternalInput")
    XvT = nc.dram_tensor("XvT", [D_MODEL, S], BF, kind="ExternalInput")
    WqT = nc.dram_tensor("WqT", [D_MODEL, DP], BF, kind="ExternalInput")
    WkT = nc.dram_tensor("WkT", [D_MODEL, DP], BF, kind="ExternalInput")
    WvT = nc.dram_tensor("WvT", [D_MODEL, DP], BF, kind="ExternalInput")
    WoT = nc.dram_tensor("WoT", [DP, D_MODEL], BF, kind="ExternalInput")
    bqs = nc.dram_tensor("bqs", [DP, 1], F32, kind="ExternalInput")
    bk = nc.dram_tensor("bk", [DP, 1], F32, kind="ExternalInput")
    bv = nc.dram_tensor("bv", [DP, 1], F32, kind="ExternalInput")
    tri = nc.dram_tensor("tri", [128, 128], F32, kind="ExternalInput")
    attn = nc.dram_tensor("attn", [H_LOC, S, S], BF, kind="ExternalOutput")
    outT = nc.dram_tensor("outT", [D_MODEL, S], F32, kind="ExternalOutput")

    NT = S // 128    # 16 row tiles
    NC_ = S // 512   # 4 column chunks

    with tile.TileContext(nc) as tc:
        with (
            tc.tile_pool(name="const", bufs=1) as constp,
            tc.tile_pool(name="wts", bufs=1) as wts,
            tc.tile_pool(name="qkv", bufs=1) as qkv,
            tc.tile_pool(name="ctxp", bufs=1) as ctxp,
        ):
            tri_sb = constp.tile([128, 128], F32, tag="tri")
            nc.sync.dma_start(tri_sb[:], tri[:])
            bq_sb = []
            bv_sb = []
            for dp in range(4):
                t = constp.tile([128, 1], F32, tag=f"bq{dp}", name=f"bq{dp}")
                nc.sync.dma_start(t[:], bqs[dp * 128:(dp + 1) * 128, :])
                bq_sb.append(t)
            bk_sb = []
            for dp in range(4):
                t = constp.tile([128, 1], F32, tag=f"bk{dp}", name=f"bk{dp}")
                nc.sync.dma_start(t[:], bk[dp * 128:(dp + 1) * 128, :])
                bk_sb.append(t)
            for dp in range(4):
                t = constp.tile([128, 1], F32, tag=f"bv{dp}", name=f"bv{dp}")
                nc.sync.dma_start(t[:], bv[dp * 128:(dp + 1) * 128, :])
                bv_sb.append(t)

            # weights: 8 contraction strips each
            w_sb = {}
            for nm, drt in (("q", WqT), ("k", WkT), ("v", WvT)):
                w_sb[nm] = []
                for dc in range(8):
                    t = wts.tile([128, DP], BF, tag=f"w{nm}{dc}", name=f"w{nm}{dc}")
                    nc.sync.dma_start(t[:], drt[dc * 128:(dc + 1) * 128, :])
                    w_sb[nm].append(t)
            wo_sb = []
            for dp in range(4):
                t = wts.tile([128, D_MODEL], BF, tag=f"wo{dp}", name=f"wo{dp}")
                nc.sync.dma_start(t[:], WoT[dp * 128:(dp + 1) * 128, :])
                wo_sb.append(t)

            # persistent activation storage
            qT_sb = [qkv.tile([128, S], BF, tag=f"qT{dp}", name=f"qT{dp}") for dp in range(4)]
            kT_sb = [qkv.tile([128, S], BF, tag=f"kT{dp}", name=f"kT{dp}") for dp in range(4)]
            v_sb = [qkv.tile([128, DP], BF, tag=f"v{st}", name=f"v{st}") for st in range(NT)]
            ctxT_sb = [ctxp.tile([128, S], BF, tag=f"ctxT{dp}", name=f"ctxT{dp}")
                       for dp in range(4)]

            # ---------------- phase A: projections ----------------
            with (
                tc.tile_pool(name="xt", bufs=2) as xtp,
                tc.tile_pool(name="psA", bufs=4, space="PSUM") as psA,
            ):
                for nm, drt in (("q", XqT), ("k", XkT), ("v", XvT)):
                    xs = []
                    for dc in range(8):
                        t = xtp.tile([128, S], BF, tag=f"xt{dc}", name=f"xt{dc}")
                        nc.sync.dma_start(t[:], drt[dc * 128:(dc + 1) * 128, :])
                        xs.append(t)
                    if nm != "v":
                        # transposed-layout projection: out (d', s)
                        dst = qT_sb if nm == "q" else kT_sb
                        bias = bq_sb if nm == "q" else bk_sb
                        for dp in range(4):
                            for sc in range(NC_):
                                ps = psA.tile([128, 512], F32, tag="psA")
                                for dc in range(8):
                                    nc.tensor.matmul(
                                        ps[:],
                                        w_sb[nm][dc][:, dp * 128:(dp + 1) * 128],
                                        xs[dc][:, sc * 512:(sc + 1) * 512],
                                        start=(dc == 0), stop=(dc == 7),
                                    )
                                nc.scalar.activation(
                                    dst[dp][:, sc * 512:(sc + 1) * 512],
                                    ps[:], IDENT, bias=bias[dp][:],
                                )
                    else:
                        # natural-layout projection: out (s, d'), no bias
                        # (v-bias folded into ctx copy; softmax rows sum to 1)
                        for st in range(NT):
                            ps = psA.tile([128, 512], F32, tag="psA")
                            for dc in range(8):
                                nc.tensor.matmul(
                                    ps[:],
                                    xs[dc][:, st * 128:(st + 1) * 128],
                                    w_sb[nm][dc][:],
                                    start=(dc == 0), stop=(dc == 7),
                                )
                            nc.vector.tensor_copy(v_sb[st][:], ps[:])

            # ---------------- phase B: attention ----------------
            with (
                tc.tile_pool(name="pp", bufs=6) as pp,
                tc.tile_pool(name="pnp", bufs=3) as pnp,
                tc.tile_pool(name="ptp", bufs=2) as ptp,
                tc.tile_pool(name="stat", bufs=6) as stat,
                tc.tile_pool(name="psS", bufs=6, space="PSUM") as psS,
                tc.tile_pool(name="psC", bufs=2, space="PSUM") as psC,
            ):
                for u in range(H_LOC):
                    dp_u, ro = u // 2, (u % 2) * 64
                    qh = qT_sb[dp_u][ro:ro + 64, :]
                    kh = kT_sb[dp_u][ro:ro + 64, :]
                    for ci in range(4):
                        ctx_ps = psC.tile([64, 512], F32, tag="ctx")
                        strips = [ptp.tile([128, 512], BF, tag=f"pt{tj}", name=f"pt{tj}")
                                  for tj in range(4 * ci + 4)]
                        for tisub in range(4):
                            ti = ci * 4 + tisub
                            nj = ti + 1
                            w_row = nj * 128
                            njc = (nj + 3) // 4
                            pn_sb = pnp.tile([128, S], BF, tag="pn")
                            pchunks, parts = [], []
                            for jc in range(njc):
                                w = min(512, w_row - jc * 512)
                                ps = psS.tile([128, 512], F32, tag="s")
                                nc.tensor.matmul(
                                    ps[:, :w],
                                    qh[:, ti * 128:(ti + 1) * 128],
                                    kh[:, jc * 512:jc * 512 + w],
                                    start=True, stop=True,
                                )
                                if (nj - 1) // 4 == jc:
                                    dcol = (nj - 1) * 128 - jc * 512
                                    nc.vector.tensor_add(
                                        ps[:, dcol:dcol + 128],
                                        ps[:, dcol:dcol + 128], tri_sb[:])
                                part = stat.tile([128, 1], F32, tag="part")
                                p_sb = pp.tile([128, 512], BF, tag="p")
                                nc.scalar.activation(
                                    p_sb[:, :w], ps[:, :w], EXP,
                                    accum_out=part[:])
                                pchunks.append(p_sb)
                                parts.append(part)
                            if njc == 1:
                                rs = parts[0]
                            else:
                                rs = stat.tile([128, 1], F32, tag="rs")
                                nc.vector.tensor_add(rs[:], parts[0][:],
                                                     parts[1][:])
                                for jc in range(2, njc):
                                    nc.vector.tensor_add(rs[:], rs[:],
                                                         parts[jc][:])
                            recip = stat.tile([128, 1], F32, tag="recip")
                            nc.vector.reciprocal(recip[:], rs[:])
                            for jc in range(njc):
                                w = min(512, w_row - jc * 512)
                                nc.vector.tensor_scalar_mul(
                                    pn_sb[:, jc * 512:jc * 512 + w],
                                    pchunks[jc][:, :w], recip[:])
                            nc.sync.dma_start(
                                attn[u, ti * 128:(ti + 1) * 128, 0:w_row],
                                pn_sb[:, :w_row])
                            for tj in range(nj):
                                nc.sync.dma_start(
                                    strips[tj][:, tisub * 128:(tisub + 1) * 128],
                                    pn_sb[:, tj * 128:(tj + 1) * 128],
                                    transpose=True)
                        # AV: accumulate ctxT for this i-chunk
                        for tj in range(4 * ci + 4):
                            q0 = max(0, tj - 4 * ci)  # first valid tisub
                            nc.tensor.matmul(
                                ctx_ps[:, q0 * 128:512],
                                v_sb[tj][:, u * 64:(u + 1) * 64],
                                strips[tj][:, q0 * 128:512],
                                start=(tj == 0), stop=(tj == 4 * ci + 3),
                            )
                        nc.scalar.activation(
                            ctxT_sb[dp_u][ro:ro + 64, ci * 512:(ci + 1) * 512],
                            ctx_ps[:], IDENT,
                            bias=bv_sb[dp_u][ro:ro + 64, :])

            # ---------------- phase C: output projection ----------------
            with (
                tc.tile_pool(name="op", bufs=3) as op,
                tc.tile_pool(name="psO", bufs=4, space="PSUM") as psO,
            ):
                for ec in range(8):
                    for sc in range(NC_):
                        ps = psO.tile([128, 512], F32, tag="o")
                        for dp in range(4):
                            nc.tensor.matmul(
                                ps[:],
                                wo_sb[dp][:, ec * 128:(ec + 1) * 128],
                                ctxT_sb[dp][:, sc * 512:(sc + 1) * 512],
                                start=(dp == 0), stop=(dp == 3),
                            )
                        o_sb = op.tile([128, 512], F32, tag="o")
                        nc.vector.tensor_copy(o_sb[:], ps[:])
                        nc.sync.dma_start(
                            outT[ec * 128:(ec + 1) * 128,
                                 sc * 512:(sc + 1) * 512], o_sb[:])

    nc.finalize()
    return nc


def _get_runner():
    if "runner" in _cache:
        return _cache["runner"]

    import jax
    from jax.sharding import Mesh, PartitionSpec
    from jax.experimental.shard_map import shard_map
    from concourse import mybir
    from concourse.bass2jax import (_bass_exec_p, install_neuronx_cc_hook,
                                    partition_id_tensor)

    install_neuronx_cc_hook()
    nc = _build_nc()

    partition_name = (nc.partition_id_tensor.name
                      if nc.partition_id_tensor else None)
    in_names, out_names, out_avals = [], [], []
    for alloc in nc.m.functions[0].allocations:
        if not isinstance(alloc, mybir.MemoryLocationSet):
            continue
        name = alloc.memorylocations[0].name
        if alloc.kind == "ExternalInput":
            if name != partition_name:
                in_names.append(name)
        elif alloc.kind == "ExternalOutput":
            out_names.append(name)
            out_avals.append(jax.core.ShapedArray(
                tuple(alloc.tensor_shape), mybir.dt.np(alloc.dtype)))
    n_params = len(in_names)
    all_names = list(in_names + out_names)
    if partition_name is not None:
        all_names.append(partition_name)
    donate = tuple(range(n_params, n_params + len(out_names)))

    def _body(*args):
        operands = list(args)
        if partition_name is not None:
            operands.append(partition_id_tensor())
        outs = _bass_exec_p.bind(
            *operands,
            out_avals=tuple(out_avals),
            in_names=tuple(all_names),
            out_names=tuple(out_names),
            lowering_input_output_aliases=(),
            sim_require_finite=True,
            sim_require_nnan=True,
            nc=nc,
        )
        return tuple(outs)

    devices = jax.devices()[:N_CORES]
    mesh = Mesh(np.asarray(devices), ("core",))
    nio = n_params + len(out_names)
    sharded = jax.jit(
        shard_map(_body, mesh=mesh,
                  in_specs=(PartitionSpec("core"),) * nio,
                  out_specs=(PartitionSpec("core"),) * len(out_names),
                  check_rep=False),
        donate_argnums=donate, keep_unused=True,
    )

    runner = {"fn": sharded, "in_names": in_names, "out_names": out_names,
              "out_avals": out_avals}
    _cache["runner"] = runner
    return runner


def _prep_inputs(Q, K, V, Wq, bq, Wk, bk, Wv, bv):
    """Build the per-core input map (host-side shard + relayout)."""
    tri_np = np.triu(np.full((128, 128), NEG, np.float32), 1)
    xqT, xkT, xvT = {}, {}, {}
    for b in range(B):
        xqT[b] = np.ascontiguousarray(Q[b].astype(BF16).T)
        xkT[b] = np.ascontiguousarray(K[b].astype(BF16).T)
        xvT[b] = np.ascontiguousarray(V[b].astype(BF16).T)
    in_maps = []
    for c in range(N_CORES):
        b, hg = c // 2, c % 2
        rows = slice(hg * DP, (hg + 1) * DP)
        in_maps.append({
            "XqT": xqT[b], "XkT": xkT[b], "XvT": xvT[b],
            "WqT": np.ascontiguousarray((Wq[rows, :] * SCALE).T).astype(BF16),
            "WkT": np.ascontiguousarray(Wk[rows, :].T).astype(BF16),
            "WvT": np.ascontiguousarray(Wv[rows, :].T).astype(BF16),
            "WoT": None,  # filled by caller (needs Wo)
            "bqs": (bq[rows] * SCALE).astype(np.float32).reshape(DP, 1),
            "bk": bk[rows].astype(np.float32).reshape(DP, 1),
            "bv": bv[rows].astype(np.float32).reshape(DP, 1),
            "tri": tri_np,
        })
    return in_maps


def _run_device(in_maps):
    import jax
    runner = _get_runner()
    fn = runner["fn"]
    concat_in = [
        np.concatenate([np.asarray(in_maps[c][n]) for c in range(N_CORES)],
                       axis=0)
        for n in runner["in_names"]
    ]
    concat_zeros = [
        np.zeros((N_CORES * a.shape[0], *a.shape[1:]), a.dtype)
        for a in runner["out_avals"]
    ]
    outs = fn(*concat_in, *concat_zeros)
    outs = [np.asarray(o) for o in jax.block_until_ready(outs)]
    results = []
    for c in range(N_CORES):
        d = {}
        for i, n in enumerate(runner["out_names"]):
            a = runner["out_avals"][i]
            d[n] = outs[i].reshape(N_CORES, *a.shape)[c]
        results.append(d)
    return results


def _numpy_fallback(Q, K, V, mask, Wq, bq, Wk, bk, Wv, bv, Wo, bo):
    """Reference math in numpy (used only if the mask is not causal)."""
    def split_heads(x):
        return x.reshape(B, S, NUM_HEADS, D_K).transpose(0, 2, 1, 3)
    q = split_heads(Q @ Wq.T + bq)
    k = split_heads(K @ Wk.T + bk)
    v = split_heads(V @ Wv.T + bv)
    out = np.empty((B, S, D_MODEL), np.float32)
    attn = np.empty((B, NUM_HEADS, S, S), np.float32)
    m = mask[0, 0]
    for b in range(B):
        for h in range(NUM_HEADS):
            s = (q[b, h] @ k[b, h].T) * SCALE
            s = np.where(m == 0, -np.inf, s)
            s = s - s.max(axis=1, keepdims=True)
            p = np.exp(s)
            p /= p.sum(axis=1, keepdims=True)
            attn[b, h] = p
            out[b, :, h * D_K:(h + 1) * D_K] = p @ v[b, h]
    out = out @ Wo.T + bo
    return out, attn


def kernel(Q, K, V, mask, Wq, bq, Wk, bk, Wv, bv, Wo, bo):
    Q = np.asarray(Q); K = np.asarray(K); V = np.asarray(V)
    mask = np.asarray(mask)
    Wq = np.asarray(Wq); bq = np.asarray(bq)
    Wk = np.asarray(Wk); bk = np.asarray(bk)
    Wv = np.asarray(Wv); bv = np.asarray(bv)
    Wo = np.asarray(Wo); bo = np.asarray(bo)

    causal = bool(np.array_equal(mask[0, 0], np.tril(np.ones((S, S),
                                                            mask.dtype))))
    if not causal:
        return _numpy_fallback(Q, K, V, mask, Wq, bq, Wk, bk, Wv, bv, Wo, bo)

    in_maps = _prep_inputs(Q, K, V, Wq, bq, Wk, bk, Wv, bv)
    for c in range(N_CORES):
        hg = c % 2
        rows = slice(hg * DP, (hg + 1) * DP)
        in_maps[c]["WoT"] = np.ascontiguousarray(Wo[:, rows].T).astype(BF16)

    results = _run_device(in_maps)

    out = np.empty((B, S, D_MODEL), np.float32)
    attn = np.empty((B, NUM_HEADS, S, S), np.float32)
    for b in range(B):
        oT = (results[2 * b]["outT"].astype(np.float32)
              + results[2 * b + 1]["outT"].astype(np.float32))
        out[b] = oT.T + bo
        attn[b, 0:H_LOC] = results[2 * b]["attn"].astype(np.float32)
        attn[b, H_LOC:] = results[2 * b + 1]["attn"].astype(np.float32)
    return out, attn
